# revision 19
# baseline (speedup 1.0000x reference)
"""BiLSTM-CRF fused Trainium2 kernel.

Strategy: data-parallel over batch (64 rows -> 8 NeuronCores x 8 rows).
A single NEFF per core runs the whole model on device:
  - xg = Wih @ emb + bias generated on the fly per 16-step block
  - fwd+bwd LSTM scans interleaved, so one direction's elementwise chain
    hides under the other direction's PE weight loads
  - emissions (small matmuls into PSUM) + Viterbi DP with a packed-group
    layout (partition p = b4*20+cur, free (g, prev)); argmax via
    is_equal/iota-encode; backtrace on host
Precision: fp16 weights/emb/xg and fp16 recurrence state, h additionally
stored f32 for the emission matmuls, Viterbi in f32 with 0/1 indicator
matmuls (exact) -> bit-identical tags to the f32 reference on the spec
inputs.

Layouts (per core, BS=8 rows):
  embT   (128, T*8)  fp16  col = t*8 + b
  wihT   (128, 2048) fp16  col = d*1024 + m*128 + j        (lhsT for xg)
  whhT_k (128, 2048) fp16  k in {0,1}: rows k*128.. of Whh_ro.T
  xg blk (128, 1024) fp16  col = m*128 + t_loc*8 + b       (per dir)
  h16/h32 (128, T*16)      col = t*16 + k*8 + b
  gate chunk order: [i0 i1 f0 f1 o0 o1 g0 g1] -> per-step psum (128,64):
     i = cols 0:16, f = 16:32, o = 32:48, g = 48:64
"""

import os
import sys
import time

sys.path.insert(0, "/opt/trn_rl_repo")

import numpy as np

B, T, E, H, V, K = 64, 512, 128, 256, 50000, 20
BS = 8
NCORES = 8
G4 = 4 * H
PERM = [0, 1, 2, 3, 6, 7, 4, 5]   # i0 i1 f0 f1 o0 o1 g0 g1
fp16 = np.float16

LAST_EXEC_TIME_NS = None
_NC_CACHE = {}


# --------------------------------------------------------------------------
# device kernel builder
# --------------------------------------------------------------------------

def _build(T, debug=False):
    import concourse.bass as bass
    from concourse import bacc, mybir
    from concourse.tile import TileContext

    KVAR = os.environ.get("KVAR", "")

    f32 = mybir.dt.float32
    fp16 = mybir.dt.float16
    u32 = mybir.dt.uint32
    AF = mybir.ActivationFunctionType
    OP = mybir.AluOpType
    AX = mybir.AxisListType

    nc = bacc.Bacc()

    d_embT = nc.dram_tensor("embT", (E, T * BS), fp16, kind="ExternalInput")
    d_wihT = nc.dram_tensor("wihT", (E, 2 * G4), fp16, kind="ExternalInput")
    d_whhT0 = nc.dram_tensor("whhT0", (128, 2 * G4), fp16, kind="ExternalInput")
    d_whhT1 = nc.dram_tensor("whhT1", (128, 2 * G4), fp16, kind="ExternalInput")
    d_bias = nc.dram_tensor("bias", (128, 16), f32, kind="ExternalInput")
    d_woutT = nc.dram_tensor("woutT", (128, 80), f32, kind="ExternalInput")
    d_Eg0 = nc.dram_tensor("Eg0", (8, 80), f32, kind="ExternalInput")
    d_Eg1 = nc.dram_tensor("Eg1", (8, 80), f32, kind="ExternalInput")
    d_indc = nc.dram_tensor("indc", (20, 80), f32, kind="ExternalInput")
    d_trdup = nc.dram_tensor("trdup", (20, 40), f32, kind="ExternalInput")
    d_iotar = nc.dram_tensor("iotar", (80, 40), f32, kind="ExternalInput")
    d_E2g0 = nc.dram_tensor("E2g0", (80, 8), f32, kind="ExternalInput")
    d_E2g1 = nc.dram_tensor("E2g1", (80, 8), f32, kind="ExternalInput")
    d_E3 = nc.dram_tensor("E3", (80, 20), f32, kind="ExternalInput")
    d_ones = nc.dram_tensor("ones18", (1, 8), f32, kind="ExternalInput")
    d_bout = nc.dram_tensor("bout", (1, 20), f32, kind="ExternalInput")
    d_start = nc.dram_tensor("start", (1, 20), f32, kind="ExternalInput")
    d_endr = nc.dram_tensor("endr", (8, 20), f32, kind="ExternalInput")

    d_hist = nc.dram_tensor("hist", (80, 2 * (T - 1)), f32, kind="ExternalOutput")
    d_last = nc.dram_tensor("last", (8, 8), u32, kind="ExternalOutput")
    if debug:
        d_hf = nc.dram_tensor("dbg_hf", (128, T * 16), f32, kind="ExternalOutput")
        d_hb = nc.dram_tensor("dbg_hb", (128, T * 16), f32, kind="ExternalOutput")
        d_S = nc.dram_tensor("dbg_S", (8, 20), f32, kind="ExternalOutput")

    with TileContext(nc) as tc:
        with (
            tc.tile_pool(name="const", bufs=1) as cp,
            tc.tile_pool(name="work", bufs=3) as wp,
            tc.tile_pool(name="xgp", bufs=1, space="PSUM") as xgp,
            tc.tile_pool(name="gp", bufs=2, space="PSUM") as gp,
            tc.tile_pool(name="candp", bufs=2, space="PSUM") as candp,
            tc.tile_pool(name="sp", bufs=1, space="PSUM") as sp,
        ):
            embT = cp.tile([E, T * BS], fp16)
            wihT = cp.tile([E, 2 * G4], fp16)
            whhT = [cp.tile([128, 2 * G4], fp16, name=f"whhT{k}", tag=f"whhT{k}")
                    for k in range(2)]
            bias = cp.tile([128, 16], f32)
            woutT = cp.tile([128, 80], f32)
            Eg0 = cp.tile([8, 80], f32)
            Eg1 = cp.tile([8, 80], f32)
            indc = cp.tile([20, 80], f32)
            trdup = cp.tile([20, 40], f32)
            iotar = cp.tile([80, 40], f32)
            E2g0 = cp.tile([80, 8], f32)
            E2g1 = cp.tile([80, 8], f32)
            E3 = cp.tile([80, 20], f32)
            ones18 = cp.tile([1, 8], f32)
            bout = cp.tile([1, 20], f32)
            start = cp.tile([1, 20], f32)
            endr = cp.tile([8, 20], f32)
            for sb, dr in [(embT, d_embT), (wihT, d_wihT), (whhT[0], d_whhT0),
                           (whhT[1], d_whhT1), (bias, d_bias), (woutT, d_woutT),
                           (Eg0, d_Eg0), (Eg1, d_Eg1), (indc, d_indc),
                           (trdup, d_trdup), (iotar, d_iotar), (E2g0, d_E2g0),
                           (E2g1, d_E2g1), (E3, d_E3), (ones18, d_ones),
                           (bout, d_bout), (start, d_start), (endr, d_endr)]:
                nc.sync.dma_start(sb[:], dr[:])

            h16 = {d: cp.tile([128, T * 16], fp16, name=f"h16_{d}",
                              tag=f"h16_{d}") for d in (0, 1)}
            h32 = {d: cp.tile([128, T * 16], f32, name=f"h32_{d}",
                              tag=f"h32_{d}") for d in (0, 1)}
            hist = cp.tile([80, 2 * (T - 1)], f32)
            S_sb = cp.tile([8, 20], f32)
            h0 = cp.tile([128, 16], fp16)
            nc.vector.memset(h0[:], 0.0)

            # DMA fence: absorb every DMA-queue wait onto cheap DVE copies so
            # later compute ops never need more than one sync wait.
            fence = cp.tile([128, 18], f32)
            for j, sb in enumerate([embT, wihT, whhT[0], whhT[1], bias, woutT,
                                    Eg0, Eg1, indc, trdup, iotar, E2g0, E2g1,
                                    E3, ones18, bout, start, endr]):
                p = sb.shape[0]
                nc.vector.tensor_copy(fence[0:p, j:j + 1], sb[0:p, 0:1])

            # ---- interleaved fwd/bwd LSTM scans with on-the-fly xg ----
            def gen_xg_block(d, blk):
                """xg for 16 steps of dir d -> (128, 1024) fp16, col m*128+t*8+b."""
                xb = wp.tile([128, 1024], fp16, name=f"xb{d}", tag=f"xb{d}")
                if d == 0:
                    c0 = blk * 128
                else:
                    c0 = T * BS - (blk + 1) * 128
                for m in range(8):
                    ps = xgp.tile([128, 128], f32, name="xgps", tag="xgps")
                    nc.tensor.matmul(
                        ps[:],
                        wihT[:, d * G4 + m * 128:d * G4 + (m + 1) * 128],
                        embT[:, c0:c0 + 128],
                        start=True, stop=True,
                    )
                    bb = bias[:, d * 8 + m:d * 8 + m + 1] \
                        .broadcast_to((128, 128))
                    nc.vector.tensor_tensor(
                        xb[:, m * 128:(m + 1) * 128], ps[:], bb, OP.add)
                return xb

            # Per-direction state and work tiles with per-direction pool
            # tags: the two recurrent chains share no tile slots, so one
            # direction's elementwise chain can run while the other
            # direction's matmuls occupy the PE (staggered overlap).
            c_st = {d: cp.tile([128, 16], f32, name=f"c{d}", tag=f"c{d}")
                    for d in (0, 1)}
            for d in (0, 1):
                nc.vector.memset(c_st[d][:], 0.0)

            def scan_step(d, i, xb):
                tt = i if d == 0 else T - 1 - i
                pt = tt - 1 if d == 0 else tt + 1
                t_loc = (i % 16) if d == 0 else 15 - (i % 16)
                gps = gp.tile([128, 64], f32, name=f"gps{d}", tag=f"gps{d}")
                for m in range(8):
                    for k in range(2):
                        o = pt * 16 + k * 8
                        rhs = (h0[:, k * 8:(k + 1) * 8]
                               if (i == 0 or KVAR == "noew")
                               else h16[d][:, o:o + 8])
                        nc.tensor.matmul(
                            gps[:, m * 8:(m + 1) * 8],
                            whhT[k][:, d * G4 + m * 128:d * G4 + (m + 1) * 128],
                            rhs,
                            start=(k == 0), stop=(k == 1),
                        )
                if KVAR == "noew":
                    return
                g_sb = wp.tile([128, 64], f32, name=f"g_sb{d}", tag=f"g_sb{d}",
                               bufs=2)
                xbv = xb[:].rearrange("p (m t b) -> p m t b", m=8, b=8)
                nc.vector.tensor_tensor(
                    g_sb[:].rearrange("p (m b) -> p m b", b=8),
                    gps[:].rearrange("p (m b) -> p m b", b=8),
                    xbv[:, :, t_loc, :], OP.add)
                acts = wp.tile([128, 64], f32, name=f"acts{d}", tag=f"acts{d}",
                               bufs=2)
                nc.scalar.activation(acts[:, 0:48], g_sb[:, 0:48], AF.Sigmoid)
                nc.scalar.activation(acts[:, 48:64], g_sb[:, 48:64], AF.Tanh)
                ig = wp.tile([128, 16], f32, name=f"ig{d}", tag=f"ig{d}",
                             bufs=2)
                fc = wp.tile([128, 16], f32, name=f"fc{d}", tag=f"fc{d}",
                             bufs=2)
                nc.vector.tensor_mul(ig[:], acts[:, 0:16], acts[:, 48:64])
                nc.vector.tensor_mul(fc[:], acts[:, 16:32], c_st[d][:])
                nc.vector.tensor_add(c_st[d][:], ig[:], fc[:])
                tc_sb = wp.tile([128, 16], f32, name=f"tc{d}", tag=f"tc{d}",
                                bufs=2)
                nc.scalar.activation(tc_sb[:], c_st[d][:], AF.Tanh)
                nc.vector.tensor_mul(
                    h16[d][:, tt * 16:(tt + 1) * 16], acts[:, 32:48], tc_sb[:])
                nc.gpsimd.tensor_copy(h32[d][:, tt * 16:(tt + 1) * 16],
                                      h16[d][:, tt * 16:(tt + 1) * 16])

            assert T % 16 == 0
            for blk in range(T // 16):
                xb1 = gen_xg_block(1, blk)
                xb0 = gen_xg_block(0, blk)
                for i16 in range(16):
                    i = blk * 16 + i16
                    scan_step(1, i, xb1)
                    scan_step(0, i, xb0)

            # ---- emissions + viterbi ----
            def em_mms(sps, t, start_flag, stop_flag):
                first = start_flag
                for d in (0, 1):
                    for k in range(2):
                        nc.tensor.matmul(
                            sps[:],
                            h32[d][:, t * 16 + k * 8:t * 16 + (k + 1) * 8],
                            woutT[:, (2 * d + k) * 20:(2 * d + k + 1) * 20],
                            start=first, stop=False, skip_group_check=True,
                        )
                        first = False
                nc.tensor.matmul(sps[:], ones18[:], bout[:],
                                 start=False, stop=stop_flag,
                                 skip_group_check=True)

            skip_vit = KVAR in ("novit", "noew")
            if skip_vit:
                nc.vector.memset(hist[:], 0.0)
                fidx = cp.tile([8, 8], mybir.dt.uint32)
                nc.vector.memset(fidx[:], 0)
            if KVAR != "noew":
                sps = sp.tile([8, 20], f32, name="sps", tag="sps")
                em_mms(sps, 0, True, False)
                nc.tensor.matmul(sps[:], ones18[:], start[:],
                                 start=False, stop=True, skip_group_check=True)
                nc.scalar.copy(S_sb[:], sps[:])

            for t in ([] if skip_vit else range(1, T)):
                cand = candp.tile([80, 40], f32, name="cand", tag="cand")
                nc.tensor.matmul(cand[:, 0:20], Eg0[:], S_sb[:],
                                 start=True, stop=False, skip_group_check=True)
                nc.tensor.matmul(cand[:, 20:40], Eg1[:], S_sb[:],
                                 start=False, stop=False, skip_group_check=True)
                nc.tensor.matmul(cand[:], indc[:], trdup[:],
                                 start=False, stop=True, skip_group_check=True)
                cand3 = cand[:].rearrange("p (g k) -> p g k", g=2)
                best2 = wp.tile([80, 2], f32, name="best2", tag="best2")
                nc.vector.tensor_reduce(best2[:], cand3, AX.X, OP.max)
                rg0 = wp.tile([80, 20], f32, name="rg0", tag="rg0")
                rg1 = wp.tile([80, 20], f32, name="rg1", tag="rg1")
                nc.vector.tensor_scalar(rg0[:], E3[:], best2[:, 0:1], None,
                                        OP.mult)
                nc.vector.tensor_scalar(rg1[:], E3[:], best2[:, 1:2], None,
                                        OP.mult)
                mask = wp.tile([80, 40], f32, name="mask", tag="mask")
                nc.vector.tensor_tensor(
                    mask[:].rearrange("p (g k) -> p g k", g=2), cand3,
                    best2[:].unsqueeze(2).broadcast_to((80, 2, 20)),
                    OP.is_equal)
                enc = wp.tile([80, 40], f32, name="enc", tag="enc")
                nc.vector.tensor_mul(enc[:], mask[:], iotar[:])
                nc.vector.tensor_reduce(
                    hist[:, (t - 1) * 2:t * 2],
                    enc[:].rearrange("p (g k) -> p g k", g=2),
                    AX.X, OP.max)
                sps = sp.tile([8, 20], f32, name="sps", tag="sps")
                nc.tensor.matmul(sps[:], E2g0[:], rg0[:],
                                 start=True, stop=False, skip_group_check=True)
                nc.tensor.matmul(sps[:], E2g1[:], rg1[:],
                                 start=False, stop=False, skip_group_check=True)
                em_mms(sps, t, False, True)
                nc.scalar.copy(S_sb[:], sps[:])

            if not skip_vit:
                Sf = cp.tile([8, 20], f32)
                nc.vector.tensor_add(Sf[:], S_sb[:], endr[:])
                fmax = cp.tile([8, 8], f32)
                fidx = cp.tile([8, 8], mybir.dt.uint32)
                nc.vector.max(fmax[:], Sf[:])
                nc.vector.max_index(fidx[:], fmax[:], Sf[:])

            nc.sync.dma_start(d_hist[:], hist[:])
            nc.sync.dma_start(d_last[:], fidx[:])
            if debug:
                nc.sync.dma_start(d_hf[:], h32[0][:])
                nc.sync.dma_start(d_hb[:], h32[1][:])
                nc.sync.dma_start(d_S[:], S_sb[:])
    nc.finalize()
    return nc


# --------------------------------------------------------------------------
# host-side packing / decode
# --------------------------------------------------------------------------

def _reorder_gates(w):
    return np.concatenate([w[p * 128:(p + 1) * 128] for p in PERM], axis=0)


def _pack_shared(Wih_f, Whh_f, b_f, Wih_b, Whh_b, b_b, Wout, bout,
                 start_trans, end_trans, transitions):
    f32 = np.float32
    out = {}
    wihT = np.concatenate(
        [_reorder_gates(Wih_f.astype(f32)).T,
         _reorder_gates(Wih_b.astype(f32)).T], axis=1)
    out["wihT"] = np.ascontiguousarray(wihT).astype(fp16)
    whhT = np.concatenate(
        [_reorder_gates(Whh_f.astype(f32)).T,
         _reorder_gates(Whh_b.astype(f32)).T], axis=1)
    out["whhT0"] = np.ascontiguousarray(whhT[:128]).astype(fp16)
    out["whhT1"] = np.ascontiguousarray(whhT[128:]).astype(fp16)
    bias = np.zeros((128, 16), f32)
    for d, b in enumerate([b_f, b_b]):
        br = _reorder_gates(b.astype(f32).reshape(G4, 1)).reshape(G4)
        for m in range(8):
            bias[:, d * 8 + m] = br[m * 128:(m + 1) * 128]
    out["bias"] = bias
    woutT = np.zeros((128, 80), f32)
    for c in range(4):
        woutT[:, c * 20:(c + 1) * 20] = \
            Wout.astype(f32)[:, c * 128:(c + 1) * 128].T
    out["woutT"] = woutT

    tr = transitions.astype(f32)
    b4 = np.arange(80) // 20
    cur = np.arange(80) % 20
    Eg0 = np.zeros((8, 80), f32)
    Eg0[b4, np.arange(80)] = 1.0
    Eg1 = np.zeros((8, 80), f32)
    Eg1[4 + b4, np.arange(80)] = 1.0
    out["Eg0"], out["Eg1"] = Eg0, Eg1
    indc = np.zeros((20, 80), f32)
    indc[cur, np.arange(80)] = 1.0
    out["indc"] = indc
    trdup = np.zeros((20, 40), f32)
    trdup[:, 0:20] = tr.T
    trdup[:, 20:40] = tr.T
    out["trdup"] = trdup
    iotar = np.tile((20.0 - np.arange(20, dtype=f32)), 2)[None, :].repeat(80, 0)
    out["iotar"] = np.ascontiguousarray(iotar)
    E2g0 = np.zeros((80, 8), f32)
    E2g0[np.arange(80), b4] = 1.0
    E2g1 = np.zeros((80, 8), f32)
    E2g1[np.arange(80), 4 + b4] = 1.0
    out["E2g0"], out["E2g1"] = E2g0, E2g1
    E3 = np.zeros((80, 20), f32)
    E3[np.arange(80), cur] = 1.0
    out["E3"] = E3
    out["ones18"] = np.ones((1, 8), f32)
    out["bout"] = bout.astype(f32).reshape(1, 20)
    out["start"] = start_trans.astype(f32).reshape(1, 20)
    out["endr"] = np.ascontiguousarray(
        end_trans.astype(f32)[None, :].repeat(8, 0))
    return out


def _decode(hist, last, T):
    henc = np.asarray(hist, np.float64).reshape(80, T - 1, 2)
    prev = np.rint(20.0 - henc).astype(np.int64).reshape(4, 20, T - 1, 2)
    hist_dec = np.empty((T - 1, 8, 20), np.int64)
    for g in range(2):
        hist_dec[:, 4 * g:4 * g + 4, :] = prev[:, :, :, g].transpose(2, 0, 1)
    tags = np.empty((8, T), np.int64)
    tags[:, T - 1] = np.asarray(last)[:, 0].astype(np.int64)
    ar = np.arange(8)
    for t in range(T - 2, -1, -1):
        tags[:, t] = hist_dec[t, ar, tags[:, t + 1]]
    return tags


# --------------------------------------------------------------------------
# numpy fallback (reference-equivalent)
# --------------------------------------------------------------------------

def _sigmoid(x):
    return 1.0 / (1.0 + np.exp(-x))


def _lstm_scan_np(xg, Whh, reverse):
    b, t, _ = xg.shape
    h = np.zeros((b, H), np.float32)
    c = np.zeros((b, H), np.float32)
    hs = np.empty((b, t, H), np.float32)
    WhhT = np.ascontiguousarray(Whh.T)
    order = range(t - 1, -1, -1) if reverse else range(t)
    for ti in order:
        g = xg[:, ti, :] + h @ WhhT
        i = _sigmoid(g[:, 0:H])
        f = _sigmoid(g[:, H:2 * H])
        gg = np.tanh(g[:, 2 * H:3 * H])
        o = _sigmoid(g[:, 3 * H:4 * H])
        c = f * c + i * gg
        h = o * np.tanh(c)
        hs[:, ti, :] = h
    return hs


def _viterbi_np(emissions, mask, start_trans, end_trans, transitions):
    b, t, k = emissions.shape
    score = start_trans[None, :] + emissions[:, 0, :]
    hist = np.empty((t - 1, b, k), np.int32)
    for ti in range(1, t):
        cand = score[:, :, None] + transitions[None, :, :] \
            + emissions[:, ti, None, :]
        best = cand.max(axis=1)
        idx = cand.argmax(axis=1).astype(np.int32)
        m = mask[:, ti]
        score = np.where(m[:, None], best, score)
        hist[ti - 1] = idx
    score = score + end_trans[None, :]
    tag = score.argmax(axis=-1).astype(np.int32)
    tags = np.empty((b, t), np.int32)
    tags[:, t - 1] = tag
    ar = np.arange(b)
    for ti in range(t - 2, -1, -1):
        prev = hist[ti][ar, tag]
        tag = np.where(mask[:, ti + 1], prev, tag)
        tags[:, ti] = tag
    return tags


def _kernel_np(x, mask, embedding, Wih_f, Whh_f, b_f, Wih_b, Whh_b, b_b,
               Wout, bout, start_trans, end_trans, transitions):
    emb = embedding[np.asarray(x, np.int64)]
    ef = emb.reshape(B * T, E).astype(np.float32)
    xg_f = (ef @ Wih_f.T).reshape(B, T, G4) + b_f[None, None, :]
    xg_b = (ef @ Wih_b.T).reshape(B, T, G4) + b_b[None, None, :]
    h_f = _lstm_scan_np(xg_f, Whh_f, reverse=False)
    h_b = _lstm_scan_np(xg_b, Whh_b, reverse=True)
    feats = np.concatenate([h_f, h_b], axis=-1)
    em = feats.reshape(B * T, 2 * H) @ Wout.T
    em = em.reshape(B, T, K) + bout
    return _viterbi_np(em, mask, start_trans, end_trans, transitions)


# --------------------------------------------------------------------------
# main entry
# --------------------------------------------------------------------------

def _get_runner():
    """Build the NEFF-backed jitted SPMD callable once and cache it."""
    if "runner" in _NC_CACHE:
        return _NC_CACHE["runner"]

    import jax
    from jax.sharding import Mesh, PartitionSpec, NamedSharding
    from jax.experimental.shard_map import shard_map
    from concourse import bass2jax, mybir
    from concourse.bass2jax import _bass_exec_p, install_neuronx_cc_hook
    from concourse.bass2jax import partition_id_tensor

    install_neuronx_cc_hook()
    nc = _build(T)

    partition_name = (nc.partition_id_tensor.name
                      if nc.partition_id_tensor else None)
    in_names, out_names, out_avals, zero_shapes = [], [], [], []
    for alloc in nc.m.functions[0].allocations:
        if not isinstance(alloc, mybir.MemoryLocationSet):
            continue
        name = alloc.memorylocations[0].name
        if alloc.kind == "ExternalInput":
            if name != partition_name:
                in_names.append(name)
        elif alloc.kind == "ExternalOutput":
            out_names.append(name)
            shape = tuple(alloc.tensor_shape)
            dtype = mybir.dt.np(alloc.dtype)
            out_avals.append(jax.core.ShapedArray(shape, dtype))
            zero_shapes.append((shape, dtype))
    n_params = len(in_names)
    all_in = list(in_names) + list(out_names)
    if partition_name is not None:
        all_in.append(partition_name)
    donate = tuple(range(n_params, n_params + len(out_names)))

    def _body(*args):
        operands = list(args)
        if partition_name is not None:
            operands.append(partition_id_tensor())
        outs = _bass_exec_p.bind(
            *operands,
            out_avals=tuple(out_avals),
            in_names=tuple(all_in),
            out_names=tuple(out_names),
            lowering_input_output_aliases=(),
            sim_require_finite=True,
            sim_require_nnan=True,
            nc=nc,
        )
        return tuple(outs)

    devices = jax.devices()[:NCORES]
    mesh = Mesh(np.asarray(devices), ("core",))
    n_outs = len(out_names)
    in_specs = (PartitionSpec("core"),) * (n_params + n_outs)
    out_specs = (PartitionSpec("core"),) * n_outs
    sharded = jax.jit(
        shard_map(_body, mesh=mesh, in_specs=in_specs, out_specs=out_specs,
                  check_rep=False),
        keep_unused=True,
    )
    shard = NamedSharding(mesh, PartitionSpec("core"))
    runner = {
        "jax": jax, "sharded": sharded, "in_names": in_names,
        "out_names": out_names, "zero_shapes": zero_shapes, "shard": shard,
    }
    _NC_CACHE["runner"] = runner
    return runner


def _run_device(x, mask, embedding, Wih_f, Whh_f, b_f, Wih_b, Whh_b, b_b,
                Wout, bout, start_trans, end_trans, transitions):
    global LAST_EXEC_TIME_NS
    r = _get_runner()
    jax = r["jax"]

    shared = _pack_shared(Wih_f, Whh_f, b_f, Wih_b, Whh_b, b_b, Wout, bout,
                          start_trans, end_trans, transitions)
    emb = embedding.astype(np.float32)[np.asarray(x, np.int64)]  # (B,T,E)
    in_maps = []
    for c in range(NCORES):
        ec = emb[c * BS:(c + 1) * BS]                        # (BS,T,E)
        embT = np.ascontiguousarray(
            ec.transpose(2, 1, 0).reshape(E, T * BS)).astype(fp16)
        in_maps.append({**shared, "embT": embT})

    concat_in = [np.concatenate([in_maps[c][nm] for c in range(NCORES)], axis=0)
                 for nm in r["in_names"]]

    def zeros():
        return [np.zeros((NCORES * s[0], *s[1:]), dt)
                for s, dt in r["zero_shapes"]]

    out_arrs = r["sharded"](*concat_in, *zeros())
    outs = {nm: np.asarray(a) for nm, a in zip(r["out_names"], out_arrs)}

    tags = np.empty((B, T), np.int32)
    for c in range(NCORES):
        hist_c = outs["hist"].reshape(NCORES, 80, -1)[c]
        last_c = outs["last"].reshape(NCORES, 8, 8)[c]
        tags[c * BS:(c + 1) * BS] = _decode(hist_c, last_c, T)

    n_time = int(os.environ.get("KERNEL_TIME_RUNS", "0"))
    if n_time > 0:
        # Device-resident inputs. The axon dispatch floor is ~60-75 ms per
        # blocking call, so single-call walls measure the RPC, not the
        # kernel. Report the per-execution slope of K pipelined dispatches
        # (launch all, block once) minus one blocking call: that is the
        # kernel-attributable device time per execution.
        K = 16
        dev_in = [jax.device_put(a, r["shard"]) for a in concat_in]
        dev_z = [jax.device_put(a, r["shard"]) for a in zeros()]
        jax.block_until_ready(dev_in)
        jax.block_until_ready(dev_z)
        t1 = None
        for _ in range(n_time):
            t0 = time.perf_counter()
            res = r["sharded"](*dev_in, *dev_z)
            jax.block_until_ready(res)
            dt = time.perf_counter() - t0
            t1 = dt if t1 is None else min(t1, dt)
        tK = None
        for _ in range(n_time):
            t0 = time.perf_counter()
            rs = [r["sharded"](*dev_in, *dev_z) for _ in range(K)]
            jax.block_until_ready(rs)
            dt = time.perf_counter() - t0
            tK = dt if tK is None else min(tK, dt)
        LAST_EXEC_TIME_NS = int(max(tK - t1, 0.0) / (K - 1) * 1e9)
        sys.stderr.write(
            f"[kernel] single-dispatch wall {t1*1e3:.1f} ms (axon floor), "
            f"per-exec slope {LAST_EXEC_TIME_NS/1e6:.3f} ms\n")
    return tags


def kernel(x, mask, embedding, Wih_f, Whh_f, b_f, Wih_b, Whh_b, b_b,
           Wout, bout, start_trans, end_trans, transitions):
    x = np.asarray(x)
    mask = np.asarray(mask).astype(bool)
    args = (x, mask, np.asarray(embedding, np.float32),
            np.asarray(Wih_f, np.float32), np.asarray(Whh_f, np.float32),
            np.asarray(b_f, np.float32), np.asarray(Wih_b, np.float32),
            np.asarray(Whh_b, np.float32), np.asarray(b_b, np.float32),
            np.asarray(Wout, np.float32), np.asarray(bout, np.float32),
            np.asarray(start_trans, np.float32),
            np.asarray(end_trans, np.float32),
            np.asarray(transitions, np.float32))
    if not mask.all():
        return _kernel_np(*args).astype(np.int32)
    try:
        return _run_device(*args).astype(np.int32)
    except Exception as e:  # pragma: no cover - safety net
        sys.stderr.write(f"[kernel] device path failed ({e!r}); numpy fallback\n")
        return _kernel_np(*args).astype(np.int32)


# revision 24
# speedup vs baseline: 1.0082x; 1.0082x over previous
"""BiLSTM-CRF fused Trainium2 kernel.

Strategy: data-parallel over batch (64 rows -> 8 NeuronCores x 8 rows).
A single NEFF per core runs the whole model on device:
  - xg = Wih @ emb + bias generated on the fly per 16-step block
  - fwd+bwd LSTM scans interleaved, so one direction's elementwise chain
    hides under the other direction's PE weight loads
  - emissions (small matmuls into PSUM) + Viterbi DP with a packed-group
    layout (partition p = b4*20+cur, free (g, prev)); argmax via
    is_equal/iota-encode; backtrace on host
Precision: fp16 weights/emb/xg and fp16 recurrence state, h additionally
stored f32 for the emission matmuls, Viterbi in f32 with 0/1 indicator
matmuls (exact) -> bit-identical tags to the f32 reference on the spec
inputs.

Layouts (per core, BS=8 rows):
  embT   (128, T*8)  fp16  col = t*8 + b
  wihT   (128, 2048) fp16  col = d*1024 + m*128 + j        (lhsT for xg)
  whhT_k (128, 2048) fp16  k in {0,1}: rows k*128.. of Whh_ro.T
  xg blk (128, 1024) fp16  col = m*128 + t_loc*8 + b       (per dir)
  h16/h32 (128, T*16)      col = t*16 + k*8 + b
  gate chunk order: [i0 i1 f0 f1 o0 o1 g0 g1] -> per-step psum (128,64):
     i = cols 0:16, f = 16:32, o = 32:48, g = 48:64
"""

import os
import sys
import time

sys.path.insert(0, "/opt/trn_rl_repo")

import numpy as np

B, T, E, H, V, K = 64, 512, 128, 256, 50000, 20
BS = 8
NCORES = 8
G4 = 4 * H
PERM = [0, 1, 2, 3, 6, 7, 4, 5]   # i0 i1 f0 f1 o0 o1 g0 g1
fp16 = np.float16

LAST_EXEC_TIME_NS = None
_NC_CACHE = {}


# --------------------------------------------------------------------------
# device kernel builder
# --------------------------------------------------------------------------

def _build(T, debug=False):
    import concourse.bass as bass
    from concourse import bacc, mybir
    from concourse.tile import TileContext

    KVAR = os.environ.get("KVAR", "")

    f32 = mybir.dt.float32
    fp16 = mybir.dt.float16
    u32 = mybir.dt.uint32
    AF = mybir.ActivationFunctionType
    OP = mybir.AluOpType
    AX = mybir.AxisListType

    nc = bacc.Bacc()

    d_embT = nc.dram_tensor("embT", (E, T * BS), fp16, kind="ExternalInput")
    d_wihT = nc.dram_tensor("wihT", (E, 2 * G4), fp16, kind="ExternalInput")
    d_whhT0 = nc.dram_tensor("whhT0", (128, 2 * G4), fp16, kind="ExternalInput")
    d_whhT1 = nc.dram_tensor("whhT1", (128, 2 * G4), fp16, kind="ExternalInput")
    d_bias = nc.dram_tensor("bias", (128, 16), f32, kind="ExternalInput")
    d_woutT = nc.dram_tensor("woutT", (128, 80), f32, kind="ExternalInput")
    d_Eg0 = nc.dram_tensor("Eg0", (8, 80), f32, kind="ExternalInput")
    d_Eg1 = nc.dram_tensor("Eg1", (8, 80), f32, kind="ExternalInput")
    d_indc = nc.dram_tensor("indc", (20, 80), f32, kind="ExternalInput")
    d_trdup = nc.dram_tensor("trdup", (20, 40), f32, kind="ExternalInput")
    d_iotar = nc.dram_tensor("iotar", (80, 40), f32, kind="ExternalInput")
    d_E2g0 = nc.dram_tensor("E2g0", (80, 8), f32, kind="ExternalInput")
    d_E2g1 = nc.dram_tensor("E2g1", (80, 8), f32, kind="ExternalInput")
    d_E3 = nc.dram_tensor("E3", (80, 20), f32, kind="ExternalInput")
    d_ones = nc.dram_tensor("ones18", (1, 8), f32, kind="ExternalInput")
    d_bout = nc.dram_tensor("bout", (1, 20), f32, kind="ExternalInput")
    d_start = nc.dram_tensor("start", (1, 20), f32, kind="ExternalInput")
    d_endr = nc.dram_tensor("endr", (8, 20), f32, kind="ExternalInput")

    d_hist = nc.dram_tensor("hist", (80, 2 * (T - 1)), f32, kind="ExternalOutput")
    d_last = nc.dram_tensor("last", (8, 8), u32, kind="ExternalOutput")
    if debug:
        d_hf = nc.dram_tensor("dbg_hf", (128, T * 16), f32, kind="ExternalOutput")
        d_hb = nc.dram_tensor("dbg_hb", (128, T * 16), f32, kind="ExternalOutput")
        d_S = nc.dram_tensor("dbg_S", (8, 20), f32, kind="ExternalOutput")

    with TileContext(nc) as tc:
        with (
            tc.tile_pool(name="const", bufs=1) as cp,
            tc.tile_pool(name="work", bufs=3) as wp,
            tc.tile_pool(name="xgp", bufs=1, space="PSUM") as xgp,
            tc.tile_pool(name="gp", bufs=2, space="PSUM") as gp,
            tc.tile_pool(name="candp", bufs=2, space="PSUM") as candp,
            tc.tile_pool(name="sp", bufs=1, space="PSUM") as sp,
        ):
            embT = cp.tile([E, T * BS], fp16)
            wihT = cp.tile([E, 2 * G4], fp16)
            whhT = [cp.tile([128, 2 * G4], fp16, name=f"whhT{k}", tag=f"whhT{k}")
                    for k in range(2)]
            bias = cp.tile([128, 16], f32)
            woutT = cp.tile([128, 80], f32)
            Eg0 = cp.tile([8, 80], f32)
            Eg1 = cp.tile([8, 80], f32)
            indc = cp.tile([20, 80], f32)
            trdup = cp.tile([20, 40], f32)
            iotar = cp.tile([80, 40], f32)
            E2g0 = cp.tile([80, 8], f32)
            E2g1 = cp.tile([80, 8], f32)
            E3 = cp.tile([80, 20], f32)
            ones18 = cp.tile([1, 8], f32)
            bout = cp.tile([1, 20], f32)
            start = cp.tile([1, 20], f32)
            endr = cp.tile([8, 20], f32)
            for sb, dr in [(embT, d_embT), (wihT, d_wihT), (whhT[0], d_whhT0),
                           (whhT[1], d_whhT1), (bias, d_bias), (woutT, d_woutT),
                           (Eg0, d_Eg0), (Eg1, d_Eg1), (indc, d_indc),
                           (trdup, d_trdup), (iotar, d_iotar), (E2g0, d_E2g0),
                           (E2g1, d_E2g1), (E3, d_E3), (ones18, d_ones),
                           (bout, d_bout), (start, d_start), (endr, d_endr)]:
                nc.sync.dma_start(sb[:], dr[:])

            h16 = {d: cp.tile([128, T * 16], fp16, name=f"h16_{d}",
                              tag=f"h16_{d}") for d in (0, 1)}
            h32 = {d: cp.tile([128, T * 16], f32, name=f"h32_{d}",
                              tag=f"h32_{d}") for d in (0, 1)}
            hist = cp.tile([80, 2 * (T - 1)], f32)
            S_sb = cp.tile([8, 20], f32)
            h0 = cp.tile([128, 16], fp16)
            nc.vector.memset(h0[:], 0.0)

            # DMA fence: absorb every DMA-queue wait onto cheap DVE copies so
            # later compute ops never need more than one sync wait.
            fence = cp.tile([128, 18], f32)
            for j, sb in enumerate([embT, wihT, whhT[0], whhT[1], bias, woutT,
                                    Eg0, Eg1, indc, trdup, iotar, E2g0, E2g1,
                                    E3, ones18, bout, start, endr]):
                p = sb.shape[0]
                nc.vector.tensor_copy(fence[0:p, j:j + 1], sb[0:p, 0:1])

            # ---- interleaved fwd/bwd LSTM scans with on-the-fly xg ----
            def gen_xg_block(d, blk):
                """xg for 16 steps of dir d -> (128, 1024) fp16, col m*128+t*8+b."""
                xb = wp.tile([128, 1024], fp16, name=f"xb{d}", tag=f"xb{d}")
                if d == 0:
                    c0 = blk * 128
                else:
                    c0 = T * BS - (blk + 1) * 128
                for m in range(8):
                    ps = xgp.tile([128, 128], f32, name="xgps", tag="xgps")
                    nc.tensor.matmul(
                        ps[:],
                        wihT[:, d * G4 + m * 128:d * G4 + (m + 1) * 128],
                        embT[:, c0:c0 + 128],
                        start=True, stop=True,
                    )
                    bb = bias[:, d * 8 + m:d * 8 + m + 1] \
                        .broadcast_to((128, 128))
                    nc.vector.tensor_tensor(
                        xb[:, m * 128:(m + 1) * 128], ps[:], bb, OP.add)
                return xb

            c_st = {d: cp.tile([128, 16], f32, name=f"c{d}", tag=f"c{d}")
                    for d in (0, 1)}
            for d in (0, 1):
                nc.vector.memset(c_st[d][:], 0.0)

            def scan_step(d, i, xb):
                tt = i if d == 0 else T - 1 - i
                pt = tt - 1 if d == 0 else tt + 1
                t_loc = (i % 16) if d == 0 else 15 - (i % 16)
                gps = gp.tile([128, 64], f32, name=f"gps{d}", tag=f"gps{d}")
                for m in range(8):
                    for k in range(2):
                        o = pt * 16 + k * 8
                        rhs = (h0[:, k * 8:(k + 1) * 8]
                               if (i == 0 or KVAR == "noew")
                               else h16[d][:, o:o + 8])
                        nc.tensor.matmul(
                            gps[:, m * 8:(m + 1) * 8],
                            whhT[k][:, d * G4 + m * 128:d * G4 + (m + 1) * 128],
                            rhs,
                            start=(k == 0), stop=(k == 1),
                        )
                if KVAR == "noew":
                    return
                g_sb = wp.tile([128, 64], f32, name=f"g_sb{d}", tag=f"g_sb{d}",
                               bufs=3)
                xbv = xb[:].rearrange("p (m t b) -> p m t b", m=8, b=8)
                nc.vector.tensor_tensor(
                    g_sb[:].rearrange("p (m b) -> p m b", b=8),
                    gps[:].rearrange("p (m b) -> p m b", b=8),
                    xbv[:, :, t_loc, :], OP.add)
                acts = wp.tile([128, 64], f32, name=f"acts{d}", tag=f"acts{d}",
                               bufs=3)
                nc.scalar.activation(acts[:, 0:48], g_sb[:, 0:48], AF.Sigmoid)
                nc.scalar.activation(acts[:, 48:64], g_sb[:, 48:64], AF.Tanh)
                ig = wp.tile([128, 16], f32, name=f"ig{d}", tag=f"ig{d}",
                             bufs=2)
                fc = wp.tile([128, 16], f32, name=f"fc{d}", tag=f"fc{d}",
                             bufs=2)
                nc.vector.tensor_mul(ig[:], acts[:, 0:16], acts[:, 48:64])
                nc.vector.tensor_mul(fc[:], acts[:, 16:32], c_st[d][:])
                nc.vector.tensor_add(c_st[d][:], ig[:], fc[:])
                tc_sb = wp.tile([128, 16], f32, name=f"tc{d}", tag=f"tc{d}",
                                bufs=2)
                nc.scalar.activation(tc_sb[:], c_st[d][:], AF.Tanh)
                nc.vector.tensor_mul(
                    h16[d][:, tt * 16:(tt + 1) * 16], acts[:, 32:48], tc_sb[:])
                nc.gpsimd.tensor_copy(h32[d][:, tt * 16:(tt + 1) * 16],
                                      h16[d][:, tt * 16:(tt + 1) * 16])

            assert T % 16 == 0
            for blk in range(T // 16):
                xb1 = gen_xg_block(1, blk)
                xb0 = gen_xg_block(0, blk)
                for i16 in range(16):
                    i = blk * 16 + i16
                    scan_step(1, i, xb1)
                    scan_step(0, i, xb0)

            # ---- emissions + viterbi ----
            def em_mms(sps, t, start_flag, stop_flag):
                first = start_flag
                for d in (0, 1):
                    for k in range(2):
                        nc.tensor.matmul(
                            sps[:],
                            h32[d][:, t * 16 + k * 8:t * 16 + (k + 1) * 8],
                            woutT[:, (2 * d + k) * 20:(2 * d + k + 1) * 20],
                            start=first, stop=False, skip_group_check=True,
                        )
                        first = False
                nc.tensor.matmul(sps[:], ones18[:], bout[:],
                                 start=False, stop=stop_flag,
                                 skip_group_check=True)

            skip_vit = KVAR in ("novit", "noew")
            if skip_vit:
                nc.vector.memset(hist[:], 0.0)
                fidx = cp.tile([8, 8], mybir.dt.uint32)
                nc.vector.memset(fidx[:], 0)
            if KVAR != "noew":
                sps = sp.tile([8, 20], f32, name="sps", tag="sps")
                em_mms(sps, 0, True, False)
                nc.tensor.matmul(sps[:], ones18[:], start[:],
                                 start=False, stop=True, skip_group_check=True)
                nc.scalar.copy(S_sb[:], sps[:])

            for t in ([] if skip_vit else range(1, T)):
                cand = candp.tile([80, 40], f32, name="cand", tag="cand")
                nc.tensor.matmul(cand[:, 0:20], Eg0[:], S_sb[:],
                                 start=True, stop=False, skip_group_check=True)
                nc.tensor.matmul(cand[:, 20:40], Eg1[:], S_sb[:],
                                 start=False, stop=False, skip_group_check=True)
                nc.tensor.matmul(cand[:], indc[:], trdup[:],
                                 start=False, stop=True, skip_group_check=True)
                cand3 = cand[:].rearrange("p (g k) -> p g k", g=2)
                best2 = wp.tile([80, 2], f32, name="best2", tag="best2")
                nc.vector.tensor_reduce(best2[:], cand3, AX.X, OP.max)
                rg0 = wp.tile([80, 20], f32, name="rg0", tag="rg0")
                rg1 = wp.tile([80, 20], f32, name="rg1", tag="rg1")
                nc.vector.tensor_scalar(rg0[:], E3[:], best2[:, 0:1], None,
                                        OP.mult)
                nc.vector.tensor_scalar(rg1[:], E3[:], best2[:, 1:2], None,
                                        OP.mult)
                mask = wp.tile([80, 40], f32, name="mask", tag="mask")
                nc.vector.tensor_tensor(
                    mask[:].rearrange("p (g k) -> p g k", g=2), cand3,
                    best2[:].unsqueeze(2).broadcast_to((80, 2, 20)),
                    OP.is_equal)
                enc = wp.tile([80, 40], f32, name="enc", tag="enc")
                nc.vector.tensor_mul(enc[:], mask[:], iotar[:])
                nc.vector.tensor_reduce(
                    hist[:, (t - 1) * 2:t * 2],
                    enc[:].rearrange("p (g k) -> p g k", g=2),
                    AX.X, OP.max)
                sps = sp.tile([8, 20], f32, name="sps", tag="sps")
                nc.tensor.matmul(sps[:], E2g0[:], rg0[:],
                                 start=True, stop=False, skip_group_check=True)
                nc.tensor.matmul(sps[:], E2g1[:], rg1[:],
                                 start=False, stop=False, skip_group_check=True)
                em_mms(sps, t, False, True)
                nc.scalar.copy(S_sb[:], sps[:])

            if not skip_vit:
                Sf = cp.tile([8, 20], f32)
                nc.vector.tensor_add(Sf[:], S_sb[:], endr[:])
                fmax = cp.tile([8, 8], f32)
                fidx = cp.tile([8, 8], mybir.dt.uint32)
                nc.vector.max(fmax[:], Sf[:])
                nc.vector.max_index(fidx[:], fmax[:], Sf[:])

            nc.sync.dma_start(d_hist[:], hist[:])
            nc.sync.dma_start(d_last[:], fidx[:])
            if debug:
                nc.sync.dma_start(d_hf[:], h32[0][:])
                nc.sync.dma_start(d_hb[:], h32[1][:])
                nc.sync.dma_start(d_S[:], S_sb[:])
    nc.finalize()
    return nc


# --------------------------------------------------------------------------
# host-side packing / decode
# --------------------------------------------------------------------------

def _reorder_gates(w):
    return np.concatenate([w[p * 128:(p + 1) * 128] for p in PERM], axis=0)


def _pack_shared(Wih_f, Whh_f, b_f, Wih_b, Whh_b, b_b, Wout, bout,
                 start_trans, end_trans, transitions):
    f32 = np.float32
    out = {}
    wihT = np.concatenate(
        [_reorder_gates(Wih_f.astype(f32)).T,
         _reorder_gates(Wih_b.astype(f32)).T], axis=1)
    out["wihT"] = np.ascontiguousarray(wihT).astype(fp16)
    whhT = np.concatenate(
        [_reorder_gates(Whh_f.astype(f32)).T,
         _reorder_gates(Whh_b.astype(f32)).T], axis=1)
    out["whhT0"] = np.ascontiguousarray(whhT[:128]).astype(fp16)
    out["whhT1"] = np.ascontiguousarray(whhT[128:]).astype(fp16)
    bias = np.zeros((128, 16), f32)
    for d, b in enumerate([b_f, b_b]):
        br = _reorder_gates(b.astype(f32).reshape(G4, 1)).reshape(G4)
        for m in range(8):
            bias[:, d * 8 + m] = br[m * 128:(m + 1) * 128]
    out["bias"] = bias
    woutT = np.zeros((128, 80), f32)
    for c in range(4):
        woutT[:, c * 20:(c + 1) * 20] = \
            Wout.astype(f32)[:, c * 128:(c + 1) * 128].T
    out["woutT"] = woutT

    tr = transitions.astype(f32)
    b4 = np.arange(80) // 20
    cur = np.arange(80) % 20
    Eg0 = np.zeros((8, 80), f32)
    Eg0[b4, np.arange(80)] = 1.0
    Eg1 = np.zeros((8, 80), f32)
    Eg1[4 + b4, np.arange(80)] = 1.0
    out["Eg0"], out["Eg1"] = Eg0, Eg1
    indc = np.zeros((20, 80), f32)
    indc[cur, np.arange(80)] = 1.0
    out["indc"] = indc
    trdup = np.zeros((20, 40), f32)
    trdup[:, 0:20] = tr.T
    trdup[:, 20:40] = tr.T
    out["trdup"] = trdup
    iotar = np.tile((20.0 - np.arange(20, dtype=f32)), 2)[None, :].repeat(80, 0)
    out["iotar"] = np.ascontiguousarray(iotar)
    E2g0 = np.zeros((80, 8), f32)
    E2g0[np.arange(80), b4] = 1.0
    E2g1 = np.zeros((80, 8), f32)
    E2g1[np.arange(80), 4 + b4] = 1.0
    out["E2g0"], out["E2g1"] = E2g0, E2g1
    E3 = np.zeros((80, 20), f32)
    E3[np.arange(80), cur] = 1.0
    out["E3"] = E3
    out["ones18"] = np.ones((1, 8), f32)
    out["bout"] = bout.astype(f32).reshape(1, 20)
    out["start"] = start_trans.astype(f32).reshape(1, 20)
    out["endr"] = np.ascontiguousarray(
        end_trans.astype(f32)[None, :].repeat(8, 0))
    return out


def _decode(hist, last, T):
    henc = np.asarray(hist, np.float64).reshape(80, T - 1, 2)
    prev = np.rint(20.0 - henc).astype(np.int64).reshape(4, 20, T - 1, 2)
    hist_dec = np.empty((T - 1, 8, 20), np.int64)
    for g in range(2):
        hist_dec[:, 4 * g:4 * g + 4, :] = prev[:, :, :, g].transpose(2, 0, 1)
    tags = np.empty((8, T), np.int64)
    tags[:, T - 1] = np.asarray(last)[:, 0].astype(np.int64)
    ar = np.arange(8)
    for t in range(T - 2, -1, -1):
        tags[:, t] = hist_dec[t, ar, tags[:, t + 1]]
    return tags


# --------------------------------------------------------------------------
# numpy fallback (reference-equivalent)
# --------------------------------------------------------------------------

def _sigmoid(x):
    return 1.0 / (1.0 + np.exp(-x))


def _lstm_scan_np(xg, Whh, reverse):
    b, t, _ = xg.shape
    h = np.zeros((b, H), np.float32)
    c = np.zeros((b, H), np.float32)
    hs = np.empty((b, t, H), np.float32)
    WhhT = np.ascontiguousarray(Whh.T)
    order = range(t - 1, -1, -1) if reverse else range(t)
    for ti in order:
        g = xg[:, ti, :] + h @ WhhT
        i = _sigmoid(g[:, 0:H])
        f = _sigmoid(g[:, H:2 * H])
        gg = np.tanh(g[:, 2 * H:3 * H])
        o = _sigmoid(g[:, 3 * H:4 * H])
        c = f * c + i * gg
        h = o * np.tanh(c)
        hs[:, ti, :] = h
    return hs


def _viterbi_np(emissions, mask, start_trans, end_trans, transitions):
    b, t, k = emissions.shape
    score = start_trans[None, :] + emissions[:, 0, :]
    hist = np.empty((t - 1, b, k), np.int32)
    for ti in range(1, t):
        cand = score[:, :, None] + transitions[None, :, :] \
            + emissions[:, ti, None, :]
        best = cand.max(axis=1)
        idx = cand.argmax(axis=1).astype(np.int32)
        m = mask[:, ti]
        score = np.where(m[:, None], best, score)
        hist[ti - 1] = idx
    score = score + end_trans[None, :]
    tag = score.argmax(axis=-1).astype(np.int32)
    tags = np.empty((b, t), np.int32)
    tags[:, t - 1] = tag
    ar = np.arange(b)
    for ti in range(t - 2, -1, -1):
        prev = hist[ti][ar, tag]
        tag = np.where(mask[:, ti + 1], prev, tag)
        tags[:, ti] = tag
    return tags


def _kernel_np(x, mask, embedding, Wih_f, Whh_f, b_f, Wih_b, Whh_b, b_b,
               Wout, bout, start_trans, end_trans, transitions):
    emb = embedding[np.asarray(x, np.int64)]
    ef = emb.reshape(B * T, E).astype(np.float32)
    xg_f = (ef @ Wih_f.T).reshape(B, T, G4) + b_f[None, None, :]
    xg_b = (ef @ Wih_b.T).reshape(B, T, G4) + b_b[None, None, :]
    h_f = _lstm_scan_np(xg_f, Whh_f, reverse=False)
    h_b = _lstm_scan_np(xg_b, Whh_b, reverse=True)
    feats = np.concatenate([h_f, h_b], axis=-1)
    em = feats.reshape(B * T, 2 * H) @ Wout.T
    em = em.reshape(B, T, K) + bout
    return _viterbi_np(em, mask, start_trans, end_trans, transitions)


# --------------------------------------------------------------------------
# main entry
# --------------------------------------------------------------------------

def _get_runner():
    """Build the NEFF-backed jitted SPMD callable once and cache it."""
    if "runner" in _NC_CACHE:
        return _NC_CACHE["runner"]

    import jax
    from jax.sharding import Mesh, PartitionSpec, NamedSharding
    from jax.experimental.shard_map import shard_map
    from concourse import bass2jax, mybir
    from concourse.bass2jax import _bass_exec_p, install_neuronx_cc_hook
    from concourse.bass2jax import partition_id_tensor

    install_neuronx_cc_hook()
    nc = _build(T)

    partition_name = (nc.partition_id_tensor.name
                      if nc.partition_id_tensor else None)
    in_names, out_names, out_avals, zero_shapes = [], [], [], []
    for alloc in nc.m.functions[0].allocations:
        if not isinstance(alloc, mybir.MemoryLocationSet):
            continue
        name = alloc.memorylocations[0].name
        if alloc.kind == "ExternalInput":
            if name != partition_name:
                in_names.append(name)
        elif alloc.kind == "ExternalOutput":
            out_names.append(name)
            shape = tuple(alloc.tensor_shape)
            dtype = mybir.dt.np(alloc.dtype)
            out_avals.append(jax.core.ShapedArray(shape, dtype))
            zero_shapes.append((shape, dtype))
    n_params = len(in_names)
    all_in = list(in_names) + list(out_names)
    if partition_name is not None:
        all_in.append(partition_name)
    donate = tuple(range(n_params, n_params + len(out_names)))

    def _body(*args):
        operands = list(args)
        if partition_name is not None:
            operands.append(partition_id_tensor())
        outs = _bass_exec_p.bind(
            *operands,
            out_avals=tuple(out_avals),
            in_names=tuple(all_in),
            out_names=tuple(out_names),
            lowering_input_output_aliases=(),
            sim_require_finite=True,
            sim_require_nnan=True,
            nc=nc,
        )
        return tuple(outs)

    devices = jax.devices()[:NCORES]
    mesh = Mesh(np.asarray(devices), ("core",))
    n_outs = len(out_names)
    in_specs = (PartitionSpec("core"),) * (n_params + n_outs)
    out_specs = (PartitionSpec("core"),) * n_outs
    sharded = jax.jit(
        shard_map(_body, mesh=mesh, in_specs=in_specs, out_specs=out_specs,
                  check_rep=False),
        keep_unused=True,
    )
    shard = NamedSharding(mesh, PartitionSpec("core"))
    runner = {
        "jax": jax, "sharded": sharded, "in_names": in_names,
        "out_names": out_names, "zero_shapes": zero_shapes, "shard": shard,
    }
    _NC_CACHE["runner"] = runner
    return runner


def _run_device(x, mask, embedding, Wih_f, Whh_f, b_f, Wih_b, Whh_b, b_b,
                Wout, bout, start_trans, end_trans, transitions):
    global LAST_EXEC_TIME_NS
    r = _get_runner()
    jax = r["jax"]

    shared = _pack_shared(Wih_f, Whh_f, b_f, Wih_b, Whh_b, b_b, Wout, bout,
                          start_trans, end_trans, transitions)
    emb = embedding.astype(np.float32)[np.asarray(x, np.int64)]  # (B,T,E)
    in_maps = []
    for c in range(NCORES):
        ec = emb[c * BS:(c + 1) * BS]                        # (BS,T,E)
        embT = np.ascontiguousarray(
            ec.transpose(2, 1, 0).reshape(E, T * BS)).astype(fp16)
        in_maps.append({**shared, "embT": embT})

    concat_in = [np.concatenate([in_maps[c][nm] for c in range(NCORES)], axis=0)
                 for nm in r["in_names"]]

    def zeros():
        return [np.zeros((NCORES * s[0], *s[1:]), dt)
                for s, dt in r["zero_shapes"]]

    out_arrs = r["sharded"](*concat_in, *zeros())
    outs = {nm: np.asarray(a) for nm, a in zip(r["out_names"], out_arrs)}

    tags = np.empty((B, T), np.int32)
    for c in range(NCORES):
        hist_c = outs["hist"].reshape(NCORES, 80, -1)[c]
        last_c = outs["last"].reshape(NCORES, 8, 8)[c]
        tags[c * BS:(c + 1) * BS] = _decode(hist_c, last_c, T)

    n_time = int(os.environ.get("KERNEL_TIME_RUNS", "0"))
    if n_time > 0:
        # Device-resident inputs. The axon dispatch floor is ~60-75 ms per
        # blocking call, so single-call walls measure the RPC, not the
        # kernel. Report the per-execution slope of K pipelined dispatches
        # (launch all, block once) minus one blocking call: that is the
        # kernel-attributable device time per execution.
        K = 16
        dev_in = [jax.device_put(a, r["shard"]) for a in concat_in]
        dev_z = [jax.device_put(a, r["shard"]) for a in zeros()]
        jax.block_until_ready(dev_in)
        jax.block_until_ready(dev_z)
        t1 = None
        for _ in range(n_time):
            t0 = time.perf_counter()
            res = r["sharded"](*dev_in, *dev_z)
            jax.block_until_ready(res)
            dt = time.perf_counter() - t0
            t1 = dt if t1 is None else min(t1, dt)
        tK = None
        for _ in range(n_time):
            t0 = time.perf_counter()
            rs = [r["sharded"](*dev_in, *dev_z) for _ in range(K)]
            jax.block_until_ready(rs)
            dt = time.perf_counter() - t0
            tK = dt if tK is None else min(tK, dt)
        LAST_EXEC_TIME_NS = int(max(tK - t1, 0.0) / (K - 1) * 1e9)
        sys.stderr.write(
            f"[kernel] single-dispatch wall {t1*1e3:.1f} ms (axon floor), "
            f"per-exec slope {LAST_EXEC_TIME_NS/1e6:.3f} ms\n")
    return tags


def kernel(x, mask, embedding, Wih_f, Whh_f, b_f, Wih_b, Whh_b, b_b,
           Wout, bout, start_trans, end_trans, transitions):
    x = np.asarray(x)
    mask = np.asarray(mask).astype(bool)
    args = (x, mask, np.asarray(embedding, np.float32),
            np.asarray(Wih_f, np.float32), np.asarray(Whh_f, np.float32),
            np.asarray(b_f, np.float32), np.asarray(Wih_b, np.float32),
            np.asarray(Whh_b, np.float32), np.asarray(b_b, np.float32),
            np.asarray(Wout, np.float32), np.asarray(bout, np.float32),
            np.asarray(start_trans, np.float32),
            np.asarray(end_trans, np.float32),
            np.asarray(transitions, np.float32))
    if not mask.all():
        return _kernel_np(*args).astype(np.int32)
    try:
        return _run_device(*args).astype(np.int32)
    except Exception as e:  # pragma: no cover - safety net
        sys.stderr.write(f"[kernel] device path failed ({e!r}); numpy fallback\n")
        return _kernel_np(*args).astype(np.int32)
